# revision 18
# baseline (speedup 1.0000x reference)
"""ColumnParallelFusedMoeLinear grouped-GEMM kernel for 8 Trainium2 NeuronCores.

Strategy (expert/token parallel):
  Tokens are sorted by expert; m_sizes gives each expert's contiguous row
  range of x.  The host splits the full token range into single-expert
  chunks (balanced m_sizes -> one expert per core) and each core computes
  y_chunk = x_chunk @ weight[e].T, with the host scattering chunk rows back
  into the full output.

  Inputs are cast to bf16 on the host (PE streams 1 column/cycle for bf16
  same as fp32r, but HBM traffic halves; K=1024 accumulation stays fp32 in
  PSUM so the result error is ~5e-3, well inside the 2e-2 gate).  The
  output is stored transposed (yT, bf16) and the host casts/transposes it
  back, halving the store traffic too.  Per-core HBM traffic ~10 MB vs a
  ~57 us matmul roofline, so the kernel is tensor-engine-bound.

  The matmul puts the weight column block [k=128, n=128] stationary and
  streams the token dim as the moving free dim, so the ragged per-expert
  token count (973..1065 here) is NOT padded up to a multiple of 128 --
  only to the SPMD-uniform m_pad.  PSUM limits the moving dim to 512 fp32
  per bank, so the token range is cut into ceil(m_pad/512) near-equal
  chunks (all >=256 wide, keeping LDWEIGHTS hidden under the stream).
"""

import math
import os

import ml_dtypes
import numpy as np

_N_CORES = 8
_P = 128
_PSUM_F = 512  # PSUM bank width in fp32 = max moving free dim per matmul

_program_cache = {}


def _m_chunks(m_pad):
    """Cut [0, m_pad) into chunks of <=512 (multiples of 8).

    The first chunk is small (256) so the first x DMA lands early and the
    PE can start the first accumulation group sooner; the rest near-equal.
    """
    if m_pad <= _PSUM_F:
        return [(0, m_pad)]
    bounds = [(0, 256)]
    rest = m_pad - 256
    n = (rest + _PSUM_F - 1) // _PSUM_F
    base = min(_PSUM_F, ((-(-rest // n) + 7) // 8) * 8)
    c = 256
    while c < m_pad:
        bounds.append((c, min(c + base, m_pad)))
        c = bounds[-1][1]
    return bounds


def _w_groups(nt_n):
    """Weight column-tile groups in consumption order: two singles, then
    doubling widths -- small DMAs early (low latency), big ones late."""
    groups = [(0, 1), (1, 2)] if nt_n > 1 else [(0, 1)]
    while groups[-1][1] < nt_n:
        g0 = groups[-1][1]
        groups.append((g0, min(2 * g0, nt_n)))
    return groups


def _build_program(m_pad, d_in, d_out, out_engine="scalar", warm_mms=10):
    import concourse.mybir as mybir
    import concourse.tile as tile
    from concourse import bacc

    kc_n = d_in // _P   # contraction chunks of 128
    nt_n = d_out // _P  # stationary weight column tiles
    chunks = _m_chunks(m_pad)
    groups = _w_groups(nt_n)

    # Hosts packs x as [p, ci, kc, m_chunk] and w as [p, g, kc, cols] so
    # every DMA is a plain 2D contiguous slice with multi-KB partition
    # lines (small-line ramp DMAs ran at ~150 GB/s, these hit line rate).
    x_off = {}
    off = 0
    for ci, (c0, c1) in enumerate(chunks):
        x_off[ci] = off
        off += kc_n * (c1 - c0)
    w_off = {}
    off = 0
    for gi, (g0, g1) in enumerate(groups):
        w_off[gi] = off
        off += kc_n * (g1 - g0) * _P

    nc = bacc.Bacc("TRN2", target_bir_lowering=False, debug=False)
    xT = nc.dram_tensor("xT", [_P, kc_n * m_pad], mybir.dt.bfloat16,
                        kind="ExternalInput")
    wT = nc.dram_tensor("wT", [_P, kc_n * d_out], mybir.dt.bfloat16,
                        kind="ExternalInput")
    yT = nc.dram_tensor("yT", [d_out, m_pad], mybir.dt.bfloat16,
                        kind="ExternalOutput")
    y3 = yT.rearrange("(nt p) m -> nt p m", p=_P)

    with tile.TileContext(nc) as tc:
        with (
            tc.tile_pool(name="xw", bufs=1) as xwpool,
            tc.tile_pool(name="out", bufs=16) as outpool,
            tc.tile_pool(name="psum", bufs=6, space="PSUM") as psumpool,
            tc.tile_pool(name="warm", bufs=1, space="PSUM") as warmpool,
        ):
            xall = xwpool.tile([_P, kc_n * m_pad], mybir.dt.bfloat16,
                               tag="xall", name="xall")
            wall = xwpool.tile([_P, kc_n * d_out], mybir.dt.bfloat16,
                               tag="wall", name="wall")

            # PE warm-up: dummy matmuls with no DMA dependency so the HAM
            # clock gate reaches 8/8 while the input ramp streams in.  DVE
            # memset (gpsimd's Q7 startup is multi-us).
            if warm_mms:
                scr = xwpool.tile([_P, 640], mybir.dt.bfloat16, tag="scr",
                                  name="scr")
                nc.vector.memset(scr[:], 0)
                psw = warmpool.tile([_P, _PSUM_F], mybir.dt.float32, tag="psw",
                                    name="psw")
                for i in range(warm_mms):
                    nc.tensor.matmul(psw[:], scr[:, 0:_P], scr[:, _P:640],
                                     start=(i == 0), stop=(i == warm_mms - 1))

            # DMA emission order == arrival order on the sync queue, in
            # consumption order with small transfers first (per-DMA
            # completion receipt ~1.5us sits on every dependency edge):
            # w n0, x c0, x c1, w n1, x c2.., then w groups of doubling size.
            def wslice(gi):
                e = w_off[gi + 1] if gi + 1 < len(groups) else kc_n * d_out
                return w_off[gi], e

            def xslice(ci):
                e = x_off[ci + 1] if ci + 1 < len(chunks) else kc_n * m_pad
                return x_off[ci], e

            order = [("w", 0), ("x", 0)]
            if len(chunks) > 1:
                order.append(("x", 1))
            if len(groups) > 1:
                order.append(("w", 1))
            order += [("x", ci) for ci in range(2, len(chunks))]
            order += [("w", gi) for gi in range(2, len(groups))]
            # w rides the sync ring, x the scalar ring: the two cascades
            # stream in parallel during the ramp (stores only join the
            # scalar ring after the first n-group, ~16us, by when x is done)
            for kind, i in order:
                s, e = wslice(i) if kind == "w" else xslice(i)
                if kind == "w":
                    nc.sync.dma_start(wall[:, s:e], wT[:, s:e])
                else:
                    nc.scalar.dma_start(xall[:, s:e], xT[:, s:e])

            out_dma = {
                "gpsimd": nc.gpsimd.dma_start,
                "scalar": nc.scalar.dma_start,
                "sync": nc.sync.dma_start,
            }[out_engine]

            nt_group = {}
            for gi, (g0, g1) in enumerate(groups):
                for nt in range(g0, g1):
                    nt_group[nt] = (gi, g0, g1)

            for nt in range(nt_n):
                gi, g0, g1 = nt_group[nt]
                ps = [psumpool.tile([_P, _PSUM_F], mybir.dt.float32, tag="ps",
                                    name=f"ps{nt}_{ci}")
                      for ci in range(len(chunks))]
                o = outpool.tile([_P, m_pad], mybir.dt.bfloat16, tag="o")
                for ci, (c0, c1) in enumerate(chunks):
                    for kc in range(kc_n):
                        woff = (w_off[gi] + kc * (g1 - g0) * _P
                                + (nt - g0) * _P)
                        nc.tensor.matmul(
                            ps[ci][:, :c1 - c0],
                            wall[:, woff:woff + _P],
                            xall[:, x_off[ci] + kc * (c1 - c0):
                                 x_off[ci] + (kc + 1) * (c1 - c0)],
                            start=(kc == 0),
                            stop=(kc == kc_n - 1),
                        )
                    nc.vector.tensor_copy(o[:, c0:c1], ps[ci][:, :c1 - c0])
                    if nt >= nt_n - 2:
                        # per-chunk stores at the end shorten the drain tail
                        out_dma(y3[nt, :, c0:c1], o[:, c0:c1])
                if nt < nt_n - 2:
                    out_dma(y3[nt], o[:])
    nc.compile()
    return nc


def _pack_x(xT_b, chunks, kc_n):
    """[d_in, m_pad] bf16 -> [128, sum_ci kc_n*len_ci] in [ci][kc][m] order."""
    v = xT_b.reshape(kc_n, _P, -1).transpose(1, 0, 2)  # [p, kc, m]
    return np.concatenate(
        [np.ascontiguousarray(v[:, :, c0:c1]).reshape(_P, -1)
         for c0, c1 in chunks], axis=1)


def _pack_w(wT_b, groups, kc_n):
    """[d_in, d_out] bf16 -> [128, kc_n*d_out] in [g][kc][cols] order."""
    v = wT_b.reshape(kc_n, _P, -1).transpose(1, 0, 2)  # [p, kc, o]
    return np.concatenate(
        [np.ascontiguousarray(v[:, :, g0 * _P:g1 * _P]).reshape(_P, -1)
         for g0, g1 in groups], axis=1)


# Largest chunk one core handles per SPMD round (SBUF-bounded).
_MAX_CHUNK = 2560


def _plan_chunks(m_sizes, T):
    """Split [0, T) into single-expert chunks, balanced by length.

    Every chunk is <= _MAX_CHUNK rows.  Returns a list of (expert, row0,
    row1) padded with empty (0, 0, 0) chunks to a multiple of _N_CORES,
    or None if there are no rows at all.
    """
    off = np.cumsum(np.asarray(m_sizes, dtype=np.int64))
    starts = np.clip(np.concatenate([[0], off[:-1]]), 0, T)
    ends = np.clip(off, 0, T)
    segs = [(e, int(starts[e]), int(ends[e]))
            for e in range(len(m_sizes)) if ends[e] > starts[e]]
    if not segs:
        return None
    lens = np.array([s1 - s0 for _, s0, s1 in segs], dtype=np.float64)
    # mandatory splits so no chunk exceeds _MAX_CHUNK, then distribute any
    # spare cores (up to the next multiple of _N_CORES) to the biggest shares
    n_chunks = np.ceil(lens / _MAX_CHUNK).astype(np.int64)
    total = int(n_chunks.sum())
    spare = (-total) % _N_CORES if total > _N_CORES else _N_CORES - total
    for _ in range(spare):
        i = int(np.argmax(lens / n_chunks))
        n_chunks[i] += 1
    chunks = []
    for (e, s0, s1), k in zip(segs, n_chunks):
        L = s1 - s0
        bounds = [s0 + (L * i) // k for i in range(int(k) + 1)]
        for i in range(int(k)):
            if bounds[i + 1] > bounds[i]:
                chunks.append((e, bounds[i], bounds[i + 1]))
    while len(chunks) % _N_CORES:
        chunks.append((0, 0, 0))
    return chunks


def kernel(x, weight, m_sizes):
    from concourse.bass_utils import run_bass_kernel_spmd

    x = np.ascontiguousarray(np.asarray(x), dtype=np.float32)
    weight = np.asarray(weight, dtype=np.float32)
    m_arr = np.asarray(m_sizes)

    T, d_in = x.shape
    E, d_out, _ = weight.shape

    y = np.zeros((T, d_out), dtype=np.float32)
    chunks = _plan_chunks(m_arr, T)
    if chunks is None:
        return y

    max_len = max(r1 - r0 for _, r0, r1 in chunks)
    m_pad = max(_P, int(math.ceil(max_len / 16)) * 16)

    out_engine = os.environ.get("MOE_OUT_ENGINE", "scalar")
    warm_mms = int(os.environ.get("MOE_WARM_MMS", "14"))
    key = (m_pad, d_in, d_out, out_engine, warm_mms)
    if key not in _program_cache:
        _program_cache[key] = _build_program(m_pad, d_in, d_out, out_engine,
                                             warm_mms)
    nc = _program_cache[key]

    bf16 = ml_dtypes.bfloat16
    kc_n = d_in // _P
    m_chunks = _m_chunks(m_pad)
    groups = _w_groups(d_out // _P)
    # weight[e].T packed, built once per expert actually used
    wT_cache = {}
    for round0 in range(0, len(chunks), _N_CORES):
        batch = chunks[round0:round0 + _N_CORES]
        in_maps = []
        for e, r0, r1 in batch:
            xT = np.zeros((d_in, m_pad), dtype=bf16)
            if r1 > r0:
                xT[:, : r1 - r0] = x[r0:r1].T.astype(bf16)
            if e not in wT_cache:
                wT_cache[e] = _pack_w(weight[e].T.astype(bf16), groups, kc_n)
            in_maps.append({"xT": _pack_x(xT, m_chunks, kc_n),
                            "wT": wT_cache[e]})

        res = run_bass_kernel_spmd(nc, in_maps, core_ids=list(range(_N_CORES)))

        for (e, r0, r1), out in zip(batch, res.results):
            if r1 > r0:
                y[r0:r1] = out["yT"][:, : r1 - r0].T.astype(np.float32)
    return y


# revision 21
# speedup vs baseline: 1.0053x; 1.0053x over previous
"""ColumnParallelFusedMoeLinear grouped-GEMM kernel for 8 Trainium2 NeuronCores.

Strategy (expert/token parallel):
  Tokens are sorted by expert; m_sizes gives each expert's contiguous row
  range of x.  The host splits the full token range into single-expert
  chunks (balanced m_sizes -> one expert per core) and each core computes
  y_chunk = x_chunk @ weight[e].T, with the host scattering chunk rows back
  into the full output.

  Inputs are cast to bf16 on the host (PE streams 1 column/cycle for bf16
  same as fp32r, but HBM traffic halves; K=1024 accumulation stays fp32 in
  PSUM so the result error is ~5e-3, well inside the 2e-2 gate).  The
  output is stored transposed (yT, bf16) and the host casts/transposes it
  back, halving the store traffic too.  Per-core HBM traffic ~10 MB vs a
  ~57 us matmul roofline, so the kernel is tensor-engine-bound.

  The matmul puts the weight column block [k=128, n=128] stationary and
  streams the token dim as the moving free dim, so the ragged per-expert
  token count (973..1065 here) is NOT padded up to a multiple of 128 --
  only to the SPMD-uniform m_pad.  PSUM limits the moving dim to 512 fp32
  per bank, so the token range is cut into ceil(m_pad/512) near-equal
  chunks (all >=256 wide, keeping LDWEIGHTS hidden under the stream).
"""

import math
import os

import ml_dtypes
import numpy as np

_N_CORES = 8
_P = 128
_PSUM_F = 512  # PSUM bank width in fp32 = max moving free dim per matmul

_program_cache = {}


def _m_chunks(m_pad):
    """Cut [0, m_pad) into chunks of <=512 (multiples of 8).

    The first chunk is small (256) so the first x DMA lands early and the
    PE can start the first accumulation group sooner; the rest near-equal.
    """
    if m_pad <= _PSUM_F:
        return [(0, m_pad)]
    bounds = [(0, 256)]
    rest = m_pad - 256
    n = (rest + _PSUM_F - 1) // _PSUM_F
    base = min(_PSUM_F, ((-(-rest // n) + 7) // 8) * 8)
    c = 256
    while c < m_pad:
        bounds.append((c, min(c + base, m_pad)))
        c = bounds[-1][1]
    return bounds


def _w_groups(nt_n):
    """Weight column-tile groups in consumption order: two singles, then
    doubling widths (capped at 4) -- small DMAs early for low latency,
    and no group so big that its completion lags phase-A consumption."""
    groups = [(0, 1), (1, 2)] if nt_n > 1 else [(0, 1)]
    while groups[-1][1] < nt_n:
        g0 = groups[-1][1]
        groups.append((g0, min(g0 + min(g0, 4), nt_n)))
    return groups


def _build_program(m_pad, d_in, d_out, out_engine="scalar", warm_mms=10):
    import concourse.mybir as mybir
    import concourse.tile as tile
    from concourse import bacc

    kc_n = d_in // _P   # contraction chunks of 128
    nt_n = d_out // _P  # stationary weight column tiles
    chunks = _m_chunks(m_pad)
    groups = _w_groups(nt_n)

    # Hosts packs x as [p, ci, kc, m_chunk] and w as [p, g, kc, cols] so
    # every DMA is a plain 2D contiguous slice with multi-KB partition
    # lines (small-line ramp DMAs ran at ~150 GB/s, these hit line rate).
    x_off = {}
    off = 0
    for ci, (c0, c1) in enumerate(chunks):
        x_off[ci] = off
        off += kc_n * (c1 - c0)
    w_off = {}
    off = 0
    for gi, (g0, g1) in enumerate(groups):
        w_off[gi] = off
        off += kc_n * (g1 - g0) * _P

    nc = bacc.Bacc("TRN2", target_bir_lowering=False, debug=False)
    xT = nc.dram_tensor("xT", [_P, kc_n * m_pad], mybir.dt.bfloat16,
                        kind="ExternalInput")
    wT = nc.dram_tensor("wT", [_P, kc_n * d_out], mybir.dt.bfloat16,
                        kind="ExternalInput")
    yT = nc.dram_tensor("yT", [d_out, m_pad], mybir.dt.bfloat16,
                        kind="ExternalOutput")
    y3 = yT.rearrange("(nt p) m -> nt p m", p=_P)

    with tile.TileContext(nc) as tc:
        with (
            tc.tile_pool(name="xw", bufs=1) as xwpool,
            tc.tile_pool(name="out", bufs=16) as outpool,
            tc.tile_pool(name="psum", bufs=6, space="PSUM") as psumpool,
            tc.tile_pool(name="warm", bufs=1, space="PSUM") as warmpool,
        ):
            xall = xwpool.tile([_P, kc_n * m_pad], mybir.dt.bfloat16,
                               tag="xall", name="xall")
            wall = xwpool.tile([_P, kc_n * d_out], mybir.dt.bfloat16,
                               tag="wall", name="wall")

            # PE warm-up: dummy matmuls with no DMA dependency so the HAM
            # clock gate reaches 8/8 while the input ramp streams in.  DVE
            # memset (gpsimd's Q7 startup is multi-us).
            if warm_mms:
                scr = xwpool.tile([_P, 640], mybir.dt.bfloat16, tag="scr",
                                  name="scr")
                nc.vector.memset(scr[:], 0)
                psw = warmpool.tile([_P, _PSUM_F], mybir.dt.float32, tag="psw",
                                    name="psw")
                for i in range(warm_mms):
                    nc.tensor.matmul(psw[:], scr[:, 0:_P], scr[:, _P:640],
                                     start=(i == 0), stop=(i == warm_mms - 1))

            # DMA emission order == arrival order on the sync queue, in
            # consumption order with small transfers first (per-DMA
            # completion receipt ~1.5us sits on every dependency edge):
            # w n0, x c0, x c1, w n1, x c2.., then w groups of doubling size.
            def wslice(gi):
                e = w_off[gi + 1] if gi + 1 < len(groups) else kc_n * d_out
                return w_off[gi], e

            def xslice(ci):
                e = x_off[ci + 1] if ci + 1 < len(chunks) else kc_n * m_pad
                return x_off[ci], e

            # Phase-A (chunk 0 over all n) consumes w fast, so w groups are
            # interleaved with the later x chunks in consumption order.
            order = [("w", 0), ("x", 0)]
            if len(groups) > 1:
                order.append(("w", 1))
            order += [("w", gi) for gi in range(2, len(groups) - 2)]
            tail_w = [("w", gi) for gi in range(max(2, len(groups) - 2),
                                                len(groups))]
            tail_x = [("x", ci) for ci in range(1, len(chunks))]
            while tail_x or tail_w:
                if tail_w:
                    order.append(tail_w.pop(0))
                if tail_x:
                    order.append(tail_x.pop(0))
            for kind, i in order:
                s, e = wslice(i) if kind == "w" else xslice(i)
                src, dst = (wT, wall) if kind == "w" else (xT, xall)
                nc.sync.dma_start(dst[:, s:e], src[:, s:e])

            out_dma = {
                "gpsimd": nc.gpsimd.dma_start,
                "scalar": nc.scalar.dma_start,
                "sync": nc.sync.dma_start,
            }[out_engine]

            nt_group = {}
            for gi, (g0, g1) in enumerate(groups):
                for nt in range(g0, g1):
                    nt_group[nt] = (gi, g0, g1)

            # Chunk-phased loop: all n-tiles on chunk 0 first (only w + the
            # small first x chunk gate the PE), then chunk 1, then chunk 2 --
            # the x remainder has a whole phase (~18us) of slack to arrive.
            otiles = [outpool.tile([_P, m_pad], mybir.dt.bfloat16, tag="o",
                                   name=f"o{nt}") for nt in range(nt_n)]
            for ci, (c0, c1) in enumerate(chunks):
                for nt in range(nt_n):
                    gi, g0, g1 = nt_group[nt]
                    ps = psumpool.tile([_P, _PSUM_F], mybir.dt.float32,
                                       tag="ps", name=f"ps{nt}_{ci}")
                    for kc in range(kc_n):
                        woff = (w_off[gi] + kc * (g1 - g0) * _P
                                + (nt - g0) * _P)
                        nc.tensor.matmul(
                            ps[:, :c1 - c0],
                            wall[:, woff:woff + _P],
                            xall[:, x_off[ci] + kc * (c1 - c0):
                                 x_off[ci] + (kc + 1) * (c1 - c0)],
                            start=(kc == 0),
                            stop=(kc == kc_n - 1),
                        )
                    nc.vector.tensor_copy(otiles[nt][:, c0:c1],
                                          ps[:, :c1 - c0])
                    out_dma(y3[nt, :, c0:c1], otiles[nt][:, c0:c1])
    nc.compile()
    return nc


def _pack_x(xT_b, chunks, kc_n):
    """[d_in, m_pad] bf16 -> [128, sum_ci kc_n*len_ci] in [ci][kc][m] order."""
    v = xT_b.reshape(kc_n, _P, -1).transpose(1, 0, 2)  # [p, kc, m]
    return np.concatenate(
        [np.ascontiguousarray(v[:, :, c0:c1]).reshape(_P, -1)
         for c0, c1 in chunks], axis=1)


def _pack_w(wT_b, groups, kc_n):
    """[d_in, d_out] bf16 -> [128, kc_n*d_out] in [g][kc][cols] order."""
    v = wT_b.reshape(kc_n, _P, -1).transpose(1, 0, 2)  # [p, kc, o]
    return np.concatenate(
        [np.ascontiguousarray(v[:, :, g0 * _P:g1 * _P]).reshape(_P, -1)
         for g0, g1 in groups], axis=1)


# Largest chunk one core handles per SPMD round (SBUF-bounded).
_MAX_CHUNK = 2560


def _plan_chunks(m_sizes, T):
    """Split [0, T) into single-expert chunks, balanced by length.

    Every chunk is <= _MAX_CHUNK rows.  Returns a list of (expert, row0,
    row1) padded with empty (0, 0, 0) chunks to a multiple of _N_CORES,
    or None if there are no rows at all.
    """
    off = np.cumsum(np.asarray(m_sizes, dtype=np.int64))
    starts = np.clip(np.concatenate([[0], off[:-1]]), 0, T)
    ends = np.clip(off, 0, T)
    segs = [(e, int(starts[e]), int(ends[e]))
            for e in range(len(m_sizes)) if ends[e] > starts[e]]
    if not segs:
        return None
    lens = np.array([s1 - s0 for _, s0, s1 in segs], dtype=np.float64)
    # mandatory splits so no chunk exceeds _MAX_CHUNK, then distribute any
    # spare cores (up to the next multiple of _N_CORES) to the biggest shares
    n_chunks = np.ceil(lens / _MAX_CHUNK).astype(np.int64)
    total = int(n_chunks.sum())
    spare = (-total) % _N_CORES if total > _N_CORES else _N_CORES - total
    for _ in range(spare):
        i = int(np.argmax(lens / n_chunks))
        n_chunks[i] += 1
    chunks = []
    for (e, s0, s1), k in zip(segs, n_chunks):
        L = s1 - s0
        bounds = [s0 + (L * i) // k for i in range(int(k) + 1)]
        for i in range(int(k)):
            if bounds[i + 1] > bounds[i]:
                chunks.append((e, bounds[i], bounds[i + 1]))
    while len(chunks) % _N_CORES:
        chunks.append((0, 0, 0))
    return chunks


def kernel(x, weight, m_sizes):
    from concourse.bass_utils import run_bass_kernel_spmd

    x = np.ascontiguousarray(np.asarray(x), dtype=np.float32)
    weight = np.asarray(weight, dtype=np.float32)
    m_arr = np.asarray(m_sizes)

    T, d_in = x.shape
    E, d_out, _ = weight.shape

    y = np.zeros((T, d_out), dtype=np.float32)
    chunks = _plan_chunks(m_arr, T)
    if chunks is None:
        return y

    max_len = max(r1 - r0 for _, r0, r1 in chunks)
    m_pad = max(_P, int(math.ceil(max_len / 16)) * 16)

    out_engine = os.environ.get("MOE_OUT_ENGINE", "scalar")
    warm_mms = int(os.environ.get("MOE_WARM_MMS", "14"))
    key = (m_pad, d_in, d_out, out_engine, warm_mms)
    if key not in _program_cache:
        _program_cache[key] = _build_program(m_pad, d_in, d_out, out_engine,
                                             warm_mms)
    nc = _program_cache[key]

    bf16 = ml_dtypes.bfloat16
    kc_n = d_in // _P
    m_chunks = _m_chunks(m_pad)
    groups = _w_groups(d_out // _P)
    # weight[e].T packed, built once per expert actually used
    wT_cache = {}
    for round0 in range(0, len(chunks), _N_CORES):
        batch = chunks[round0:round0 + _N_CORES]
        in_maps = []
        for e, r0, r1 in batch:
            xT = np.zeros((d_in, m_pad), dtype=bf16)
            if r1 > r0:
                xT[:, : r1 - r0] = x[r0:r1].T.astype(bf16)
            if e not in wT_cache:
                wT_cache[e] = _pack_w(weight[e].T.astype(bf16), groups, kc_n)
            in_maps.append({"xT": _pack_x(xT, m_chunks, kc_n),
                            "wT": wT_cache[e]})

        res = run_bass_kernel_spmd(nc, in_maps, core_ids=list(range(_N_CORES)))

        for (e, r0, r1), out in zip(batch, res.results):
            if r1 > r0:
                y[r0:r1] = out["yT"][:, : r1 - r0].T.astype(np.float32)
    return y


# revision 25
# speedup vs baseline: 1.0419x; 1.0365x over previous
"""ColumnParallelFusedMoeLinear grouped-GEMM kernel for 8 Trainium2 NeuronCores.

Strategy (expert/token parallel):
  Tokens are sorted by expert; m_sizes gives each expert's contiguous row
  range of x.  The host splits the full token range into single-expert
  chunks (balanced m_sizes -> one expert per core) and each core computes
  y_chunk = x_chunk @ weight[e].T, with the host scattering chunk rows back
  into the full output.

  Inputs are cast to bf16 on the host (PE streams 1 column/cycle for bf16
  same as fp32r, but HBM traffic halves; K=1024 accumulation stays fp32 in
  PSUM so the result error is ~5e-3, well inside the 2e-2 gate).  The
  output is stored transposed (yT, bf16) and the host casts/transposes it
  back, halving the store traffic too.  Per-core HBM traffic ~10 MB vs a
  ~57 us matmul roofline, so the kernel is tensor-engine-bound.

  The matmul puts the weight column block [k=128, n=128] stationary and
  streams the token dim as the moving free dim, so the ragged per-expert
  token count (973..1065 here) is NOT padded up to a multiple of 128 --
  only to the SPMD-uniform m_pad.  PSUM limits the moving dim to 512 fp32
  per bank, so the token range is cut into ceil(m_pad/512) near-equal
  chunks (all >=256 wide, keeping LDWEIGHTS hidden under the stream).
"""

import math
import os

import ml_dtypes
import numpy as np

_N_CORES = 8
_P = 128
_PSUM_F = 512  # PSUM bank width in fp32 = max moving free dim per matmul

_program_cache = {}


def _m_chunks(m_pad):
    """Cut [0, m_pad) into chunks of <=512 (multiples of 8).

    The first chunk is small (256) so the first x DMA lands early and the
    PE can start the first accumulation group sooner; the rest near-equal.
    """
    if m_pad <= _PSUM_F:
        return [(0, m_pad)]
    bounds = [(0, 256)]
    rest = m_pad - 256
    n = (rest + _PSUM_F - 1) // _PSUM_F
    base = min(_PSUM_F, ((-(-rest // n) + 7) // 8) * 8)
    c = 256
    while c < m_pad:
        bounds.append((c, min(c + base, m_pad)))
        c = bounds[-1][1]
    return bounds


def _w_groups(nt_n):
    """Weight column-tile groups in consumption order: two singles, then
    doubling widths -- small DMAs early (low latency), big ones late."""
    groups = [(0, 1), (1, 2)] if nt_n > 1 else [(0, 1)]
    while groups[-1][1] < nt_n:
        g0 = groups[-1][1]
        groups.append((g0, min(2 * g0, nt_n)))
    return groups


def _build_program(m_pad, d_in, d_out, out_engine="scalar", warm_mms=10):
    import concourse.mybir as mybir
    import concourse.tile as tile
    from concourse import bacc

    kc_n = d_in // _P   # contraction chunks of 128
    nt_n = d_out // _P  # stationary weight column tiles
    chunks = _m_chunks(m_pad)
    groups = _w_groups(nt_n)

    # Hosts packs x as [p, ci, kc, m_chunk] and w as [p, g, kc, cols] so
    # every DMA is a plain 2D contiguous slice with multi-KB partition
    # lines (small-line ramp DMAs ran at ~150 GB/s, these hit line rate).
    x_off = {}
    off = 0
    for ci, (c0, c1) in enumerate(chunks):
        x_off[ci] = off
        off += kc_n * (c1 - c0)
    w_off = {}
    off = 0
    for gi, (g0, g1) in enumerate(groups):
        w_off[gi] = off
        off += kc_n * (g1 - g0) * _P

    nc = bacc.Bacc("TRN2", target_bir_lowering=False, debug=False)
    xT = nc.dram_tensor("xT", [_P, kc_n * m_pad], mybir.dt.bfloat16,
                        kind="ExternalInput")
    wT = nc.dram_tensor("wT", [_P, kc_n * d_out], mybir.dt.bfloat16,
                        kind="ExternalInput")
    yT = nc.dram_tensor("yT", [d_out, m_pad], mybir.dt.bfloat16,
                        kind="ExternalOutput")
    y3 = yT.rearrange("(nt p) m -> nt p m", p=_P)

    with tile.TileContext(nc) as tc:
        with (
            tc.tile_pool(name="xw", bufs=1) as xwpool,
            tc.tile_pool(name="out", bufs=16) as outpool,
            tc.tile_pool(name="psum", bufs=6, space="PSUM") as psumpool,
            tc.tile_pool(name="warm", bufs=1, space="PSUM") as warmpool,
        ):
            xall = xwpool.tile([_P, kc_n * m_pad], mybir.dt.bfloat16,
                               tag="xall", name="xall")
            wall = xwpool.tile([_P, kc_n * d_out], mybir.dt.bfloat16,
                               tag="wall", name="wall")

            # PE warm-up: dummy matmuls with no DMA dependency so the HAM
            # clock gate reaches 8/8 while the input ramp streams in.  DVE
            # memset (gpsimd's Q7 startup is multi-us).
            if warm_mms:
                scr = xwpool.tile([_P, 640], mybir.dt.bfloat16, tag="scr",
                                  name="scr")
                nc.vector.memset(scr[:], 0)
                psw = warmpool.tile([_P, _PSUM_F], mybir.dt.float32, tag="psw",
                                    name="psw")
                for i in range(warm_mms):
                    nc.tensor.matmul(psw[:], scr[:, 0:_P], scr[:, _P:640],
                                     start=(i == 0), stop=(i == warm_mms - 1))

            # DMA emission order == arrival order on the sync queue, in
            # consumption order with small transfers first (per-DMA
            # completion receipt ~1.5us sits on every dependency edge):
            # w n0, x c0, x c1, w n1, x c2.., then w groups of doubling size.
            def wslice(gi):
                e = w_off[gi + 1] if gi + 1 < len(groups) else kc_n * d_out
                return w_off[gi], e

            def xslice(ci):
                e = x_off[ci + 1] if ci + 1 < len(chunks) else kc_n * m_pad
                return x_off[ci], e

            order = [("w", 0), ("x", 0)]
            if len(chunks) > 1:
                order.append(("x", 1))
            if len(groups) > 1:
                order.append(("w", 1))
            order += [("x", ci) for ci in range(2, len(chunks))]
            order += [("w", gi) for gi in range(2, len(groups))]
            for kind, i in order:
                s, e = wslice(i) if kind == "w" else xslice(i)
                src, dst = (wT, wall) if kind == "w" else (xT, xall)
                nc.sync.dma_start(dst[:, s:e], src[:, s:e])

            out_dma = {
                "gpsimd": nc.gpsimd.dma_start,
                "scalar": nc.scalar.dma_start,
                "sync": nc.sync.dma_start,
            }[out_engine]

            nt_group = {}
            for gi, (g0, g1) in enumerate(groups):
                for nt in range(g0, g1):
                    nt_group[nt] = (gi, g0, g1)

            for nt in range(nt_n):
                gi, g0, g1 = nt_group[nt]
                ps = [psumpool.tile([_P, _PSUM_F], mybir.dt.float32, tag="ps",
                                    name=f"ps{nt}_{ci}")
                      for ci in range(len(chunks))]
                o = outpool.tile([_P, m_pad], mybir.dt.bfloat16, tag="o")
                for ci, (c0, c1) in enumerate(chunks):
                    for kc in range(kc_n):
                        woff = (w_off[gi] + kc * (g1 - g0) * _P
                                + (nt - g0) * _P)
                        nc.tensor.matmul(
                            ps[ci][:, :c1 - c0],
                            wall[:, woff:woff + _P],
                            xall[:, x_off[ci] + kc * (c1 - c0):
                                 x_off[ci] + (kc + 1) * (c1 - c0)],
                            start=(kc == 0),
                            stop=(kc == kc_n - 1),
                        )
                    nc.vector.tensor_copy(o[:, c0:c1], ps[ci][:, :c1 - c0])
                    if nt >= nt_n - 2:
                        # per-chunk stores at the end shorten the drain tail
                        out_dma(y3[nt, :, c0:c1], o[:, c0:c1])
                if nt < nt_n - 2:
                    out_dma(y3[nt], o[:])
    nc.compile()
    return nc


def _pack_x(xT_b, chunks, kc_n):
    """[d_in, m_pad] bf16 -> [128, sum_ci kc_n*len_ci] in [ci][kc][m] order."""
    v = xT_b.reshape(kc_n, _P, -1).transpose(1, 0, 2)  # [p, kc, m]
    return np.concatenate(
        [np.ascontiguousarray(v[:, :, c0:c1]).reshape(_P, -1)
         for c0, c1 in chunks], axis=1)


def _pack_w(wT_b, groups, kc_n):
    """[d_in, d_out] bf16 -> [128, kc_n*d_out] in [g][kc][cols] order."""
    v = wT_b.reshape(kc_n, _P, -1).transpose(1, 0, 2)  # [p, kc, o]
    return np.concatenate(
        [np.ascontiguousarray(v[:, :, g0 * _P:g1 * _P]).reshape(_P, -1)
         for g0, g1 in groups], axis=1)


# Largest chunk one core handles per SPMD round (SBUF-bounded).
_MAX_CHUNK = 2560


def _plan_chunks(m_sizes, T):
    """Split [0, T) into single-expert chunks, balanced by length.

    Every chunk is <= _MAX_CHUNK rows.  Returns a list of (expert, row0,
    row1) padded with empty (0, 0, 0) chunks to a multiple of _N_CORES,
    or None if there are no rows at all.
    """
    off = np.cumsum(np.asarray(m_sizes, dtype=np.int64))
    starts = np.clip(np.concatenate([[0], off[:-1]]), 0, T)
    ends = np.clip(off, 0, T)
    segs = [(e, int(starts[e]), int(ends[e]))
            for e in range(len(m_sizes)) if ends[e] > starts[e]]
    if not segs:
        return None
    lens = np.array([s1 - s0 for _, s0, s1 in segs], dtype=np.float64)
    # mandatory splits so no chunk exceeds _MAX_CHUNK, then distribute any
    # spare cores (up to the next multiple of _N_CORES) to the biggest shares
    n_chunks = np.ceil(lens / _MAX_CHUNK).astype(np.int64)
    total = int(n_chunks.sum())
    spare = (-total) % _N_CORES if total > _N_CORES else _N_CORES - total
    for _ in range(spare):
        i = int(np.argmax(lens / n_chunks))
        n_chunks[i] += 1
    chunks = []
    for (e, s0, s1), k in zip(segs, n_chunks):
        L = s1 - s0
        bounds = [s0 + (L * i) // k for i in range(int(k) + 1)]
        for i in range(int(k)):
            if bounds[i + 1] > bounds[i]:
                chunks.append((e, bounds[i], bounds[i + 1]))
    while len(chunks) % _N_CORES:
        chunks.append((0, 0, 0))
    return chunks


def kernel(x, weight, m_sizes):
    from concourse.bass_utils import run_bass_kernel_spmd

    x = np.ascontiguousarray(np.asarray(x), dtype=np.float32)
    weight = np.asarray(weight, dtype=np.float32)
    m_arr = np.asarray(m_sizes)

    T, d_in = x.shape
    E, d_out, _ = weight.shape

    y = np.zeros((T, d_out), dtype=np.float32)
    chunks = _plan_chunks(m_arr, T)
    if chunks is None:
        return y

    max_len = max(r1 - r0 for _, r0, r1 in chunks)
    m_pad = max(_P, int(math.ceil(max_len / 16)) * 16)

    out_engine = os.environ.get("MOE_OUT_ENGINE", "scalar")
    warm_mms = int(os.environ.get("MOE_WARM_MMS", "12"))
    key = (m_pad, d_in, d_out, out_engine, warm_mms)
    if key not in _program_cache:
        _program_cache[key] = _build_program(m_pad, d_in, d_out, out_engine,
                                             warm_mms)
    nc = _program_cache[key]

    bf16 = ml_dtypes.bfloat16
    kc_n = d_in // _P
    m_chunks = _m_chunks(m_pad)
    groups = _w_groups(d_out // _P)
    # weight[e].T packed, built once per expert actually used
    wT_cache = {}
    for round0 in range(0, len(chunks), _N_CORES):
        batch = chunks[round0:round0 + _N_CORES]
        in_maps = []
        for e, r0, r1 in batch:
            xT = np.zeros((d_in, m_pad), dtype=bf16)
            if r1 > r0:
                xT[:, : r1 - r0] = x[r0:r1].T.astype(bf16)
            if e not in wT_cache:
                wT_cache[e] = _pack_w(weight[e].T.astype(bf16), groups, kc_n)
            in_maps.append({"xT": _pack_x(xT, m_chunks, kc_n),
                            "wT": wT_cache[e]})

        res = run_bass_kernel_spmd(nc, in_maps, core_ids=list(range(_N_CORES)))

        for (e, r0, r1), out in zip(batch, res.results):
            if r1 > r0:
                y[r0:r1] = out["yT"][:, : r1 - r0].T.astype(np.float32)
    return y


# revision 28
# speedup vs baseline: 1.0646x; 1.0217x over previous
"""ColumnParallelFusedMoeLinear grouped-GEMM kernel for 8 Trainium2 NeuronCores.

Strategy (expert/token parallel):
  Tokens are sorted by expert; m_sizes gives each expert's contiguous row
  range of x.  The host splits the full token range into single-expert
  chunks (balanced m_sizes -> one expert per core) and each core computes
  y_chunk = x_chunk @ weight[e].T, with the host scattering chunk rows back
  into the full output.

  Inputs are cast to bf16 on the host (PE streams 1 column/cycle for bf16
  same as fp32r, but HBM traffic halves; K=1024 accumulation stays fp32 in
  PSUM so the result error is ~5e-3, well inside the 2e-2 gate).  The
  output is stored transposed (yT, bf16) and the host casts/transposes it
  back, halving the store traffic too.  Per-core HBM traffic ~10 MB vs a
  ~57 us matmul roofline, so the kernel is tensor-engine-bound.

  The matmul puts the weight column block [k=128, n=128] stationary and
  streams the token dim as the moving free dim, so the ragged per-expert
  token count (973..1065 here) is NOT padded up to a multiple of 128 --
  only to the SPMD-uniform m_pad.  PSUM limits the moving dim to 512 fp32
  per bank, so the token range is cut into ceil(m_pad/512) near-equal
  chunks (all >=256 wide, keeping LDWEIGHTS hidden under the stream).
"""

import math
import os

import ml_dtypes
import numpy as np

_N_CORES = 8
_P = 128
_PSUM_F = 512  # PSUM bank width in fp32 = max moving free dim per matmul

_program_cache = {}


def _m_chunks(m_pad):
    """Cut [0, m_pad) into chunks of <=512 (multiples of 8).

    The first chunk is small (256) so the first x DMA lands early and the
    PE can start the first accumulation group sooner; the rest near-equal.
    """
    if m_pad <= _PSUM_F:
        return [(0, m_pad)]
    bounds = [(0, 256)]
    rest = m_pad - 256
    n = (rest + _PSUM_F - 1) // _PSUM_F
    base = min(_PSUM_F, ((-(-rest // n) + 7) // 8) * 8)
    c = 256
    while c < m_pad:
        bounds.append((c, min(c + base, m_pad)))
        c = bounds[-1][1]
    return bounds


def _w_groups(nt_n):
    """Weight column-tile groups in consumption order: two singles, then
    doubling widths -- small DMAs early (low latency), big ones late."""
    groups = [(0, 1), (1, 2)] if nt_n > 1 else [(0, 1)]
    while groups[-1][1] < nt_n:
        g0 = groups[-1][1]
        groups.append((g0, min(2 * g0, nt_n)))
    return groups


def _build_program(m_pad, d_in, d_out, out_engine="scalar", warm_mms=10):
    import concourse.mybir as mybir
    import concourse.tile as tile
    from concourse import bacc

    kc_n = d_in // _P   # contraction chunks of 128
    nt_n = d_out // _P  # stationary weight column tiles
    chunks = _m_chunks(m_pad)
    groups = _w_groups(nt_n)

    # Hosts packs x as [p, ci, kc, m_chunk] and w as [p, g, kc, cols] so
    # every DMA is a plain 2D contiguous slice with multi-KB partition
    # lines (small-line ramp DMAs ran at ~150 GB/s, these hit line rate).
    x_off = {}
    off = 0
    for ci, (c0, c1) in enumerate(chunks):
        x_off[ci] = off
        off += kc_n * (c1 - c0)
    w_off = {}
    off = 0
    for gi, (g0, g1) in enumerate(groups):
        w_off[gi] = off
        off += kc_n * (g1 - g0) * _P

    nc = bacc.Bacc("TRN2", target_bir_lowering=False, debug=False)
    xT = nc.dram_tensor("xT", [_P, kc_n * m_pad], mybir.dt.bfloat16,
                        kind="ExternalInput")
    wT = nc.dram_tensor("wT", [_P, kc_n * d_out], mybir.dt.bfloat16,
                        kind="ExternalInput")
    yT = nc.dram_tensor("yT", [d_out, m_pad], mybir.dt.bfloat16,
                        kind="ExternalOutput")
    y3 = yT.rearrange("(nt p) m -> nt p m", p=_P)

    with tile.TileContext(nc) as tc:
        with (
            tc.tile_pool(name="xw", bufs=1) as xwpool,
            tc.tile_pool(name="out", bufs=16) as outpool,
            tc.tile_pool(name="psum", bufs=7, space="PSUM") as psumpool,
            tc.tile_pool(name="warm", bufs=1, space="PSUM") as warmpool,
        ):
            xall = xwpool.tile([_P, kc_n * m_pad], mybir.dt.bfloat16,
                               tag="xall", name="xall")
            wall = xwpool.tile([_P, kc_n * d_out], mybir.dt.bfloat16,
                               tag="wall", name="wall")

            # PE warm-up: dummy matmuls with no DMA dependency so the HAM
            # clock gate reaches 8/8 while the input ramp streams in.  DVE
            # memset (gpsimd's Q7 startup is multi-us).
            if warm_mms:
                scr = xwpool.tile([_P, 640], mybir.dt.bfloat16, tag="scr",
                                  name="scr")
                nc.vector.memset(scr[:], 0)
                psw = warmpool.tile([_P, _PSUM_F], mybir.dt.float32, tag="psw",
                                    name="psw")
                for i in range(warm_mms):
                    nc.tensor.matmul(psw[:], scr[:, 0:_P], scr[:, _P:640],
                                     start=(i == 0), stop=(i == warm_mms - 1))

            # DMA emission order == arrival order on the sync queue, in
            # consumption order with small transfers first (per-DMA
            # completion receipt ~1.5us sits on every dependency edge):
            # w n0, x c0, x c1, w n1, x c2.., then w groups of doubling size.
            def wslice(gi):
                e = w_off[gi + 1] if gi + 1 < len(groups) else kc_n * d_out
                return w_off[gi], e

            def xslice(ci):
                e = x_off[ci + 1] if ci + 1 < len(chunks) else kc_n * m_pad
                return x_off[ci], e

            order = [("w", 0), ("x", 0)]
            order += [("x", ci) for ci in range(1, len(chunks))]
            order += [("w", gi) for gi in range(1, len(groups))]
            for kind, i in order:
                s, e = wslice(i) if kind == "w" else xslice(i)
                src, dst = (wT, wall) if kind == "w" else (xT, xall)
                nc.sync.dma_start(dst[:, s:e], src[:, s:e])

            out_dma = {
                "gpsimd": nc.gpsimd.dma_start,
                "scalar": nc.scalar.dma_start,
                "sync": nc.sync.dma_start,
            }[out_engine]

            nt_group = {}
            for gi, (g0, g1) in enumerate(groups):
                for nt in range(g0, g1):
                    nt_group[nt] = (gi, g0, g1)

            for nt in range(nt_n):
                gi, g0, g1 = nt_group[nt]
                ps = [psumpool.tile([_P, _PSUM_F], mybir.dt.float32, tag="ps",
                                    name=f"ps{nt}_{ci}")
                      for ci in range(len(chunks))]
                o = outpool.tile([_P, m_pad], mybir.dt.bfloat16, tag="o")
                for ci, (c0, c1) in enumerate(chunks):
                    for kc in range(kc_n):
                        woff = (w_off[gi] + kc * (g1 - g0) * _P
                                + (nt - g0) * _P)
                        nc.tensor.matmul(
                            ps[ci][:, :c1 - c0],
                            wall[:, woff:woff + _P],
                            xall[:, x_off[ci] + kc * (c1 - c0):
                                 x_off[ci] + (kc + 1) * (c1 - c0)],
                            start=(kc == 0),
                            stop=(kc == kc_n - 1),
                        )
                    nc.vector.tensor_copy(o[:, c0:c1], ps[ci][:, :c1 - c0])
                    if nt >= nt_n - 2:
                        # per-chunk stores at the end shorten the drain tail
                        out_dma(y3[nt, :, c0:c1], o[:, c0:c1])
                if nt < nt_n - 2:
                    out_dma(y3[nt], o[:])
    nc.compile()
    return nc


def _pack_x(xT_b, chunks, kc_n):
    """[d_in, m_pad] bf16 -> [128, sum_ci kc_n*len_ci] in [ci][kc][m] order."""
    v = xT_b.reshape(kc_n, _P, -1).transpose(1, 0, 2)  # [p, kc, m]
    return np.concatenate(
        [np.ascontiguousarray(v[:, :, c0:c1]).reshape(_P, -1)
         for c0, c1 in chunks], axis=1)


def _pack_w(wT_b, groups, kc_n):
    """[d_in, d_out] bf16 -> [128, kc_n*d_out] in [g][kc][cols] order."""
    v = wT_b.reshape(kc_n, _P, -1).transpose(1, 0, 2)  # [p, kc, o]
    return np.concatenate(
        [np.ascontiguousarray(v[:, :, g0 * _P:g1 * _P]).reshape(_P, -1)
         for g0, g1 in groups], axis=1)


# Largest chunk one core handles per SPMD round (SBUF-bounded).
_MAX_CHUNK = 2560


def _plan_chunks(m_sizes, T):
    """Split [0, T) into single-expert chunks, balanced by length.

    Every chunk is <= _MAX_CHUNK rows.  Returns a list of (expert, row0,
    row1) padded with empty (0, 0, 0) chunks to a multiple of _N_CORES,
    or None if there are no rows at all.
    """
    off = np.cumsum(np.asarray(m_sizes, dtype=np.int64))
    starts = np.clip(np.concatenate([[0], off[:-1]]), 0, T)
    ends = np.clip(off, 0, T)
    segs = [(e, int(starts[e]), int(ends[e]))
            for e in range(len(m_sizes)) if ends[e] > starts[e]]
    if not segs:
        return None
    lens = np.array([s1 - s0 for _, s0, s1 in segs], dtype=np.float64)
    # mandatory splits so no chunk exceeds _MAX_CHUNK, then distribute any
    # spare cores (up to the next multiple of _N_CORES) to the biggest shares
    n_chunks = np.ceil(lens / _MAX_CHUNK).astype(np.int64)
    total = int(n_chunks.sum())
    spare = (-total) % _N_CORES if total > _N_CORES else _N_CORES - total
    for _ in range(spare):
        i = int(np.argmax(lens / n_chunks))
        n_chunks[i] += 1
    chunks = []
    for (e, s0, s1), k in zip(segs, n_chunks):
        L = s1 - s0
        bounds = [s0 + (L * i) // k for i in range(int(k) + 1)]
        for i in range(int(k)):
            if bounds[i + 1] > bounds[i]:
                chunks.append((e, bounds[i], bounds[i + 1]))
    while len(chunks) % _N_CORES:
        chunks.append((0, 0, 0))
    return chunks


def kernel(x, weight, m_sizes):
    from concourse.bass_utils import run_bass_kernel_spmd

    x = np.ascontiguousarray(np.asarray(x), dtype=np.float32)
    weight = np.asarray(weight, dtype=np.float32)
    m_arr = np.asarray(m_sizes)

    T, d_in = x.shape
    E, d_out, _ = weight.shape

    y = np.zeros((T, d_out), dtype=np.float32)
    chunks = _plan_chunks(m_arr, T)
    if chunks is None:
        return y

    max_len = max(r1 - r0 for _, r0, r1 in chunks)
    m_pad = max(_P, int(math.ceil(max_len / 4)) * 4)

    out_engine = os.environ.get("MOE_OUT_ENGINE", "scalar")
    warm_mms = int(os.environ.get("MOE_WARM_MMS", "12"))
    key = (m_pad, d_in, d_out, out_engine, warm_mms)
    if key not in _program_cache:
        _program_cache[key] = _build_program(m_pad, d_in, d_out, out_engine,
                                             warm_mms)
    nc = _program_cache[key]

    bf16 = ml_dtypes.bfloat16
    kc_n = d_in // _P
    m_chunks = _m_chunks(m_pad)
    groups = _w_groups(d_out // _P)
    # weight[e].T packed, built once per expert actually used
    wT_cache = {}
    for round0 in range(0, len(chunks), _N_CORES):
        batch = chunks[round0:round0 + _N_CORES]
        in_maps = []
        for e, r0, r1 in batch:
            xT = np.zeros((d_in, m_pad), dtype=bf16)
            if r1 > r0:
                xT[:, : r1 - r0] = x[r0:r1].T.astype(bf16)
            if e not in wT_cache:
                wT_cache[e] = _pack_w(weight[e].T.astype(bf16), groups, kc_n)
            in_maps.append({"xT": _pack_x(xT, m_chunks, kc_n),
                            "wT": wT_cache[e]})

        res = run_bass_kernel_spmd(nc, in_maps, core_ids=list(range(_N_CORES)))

        for (e, r0, r1), out in zip(batch, res.results):
            if r1 > r0:
                y[r0:r1] = out["yT"][:, : r1 - r0].T.astype(np.float32)
    return y


# revision 29
# speedup vs baseline: 1.0739x; 1.0087x over previous
"""ColumnParallelFusedMoeLinear grouped-GEMM kernel for 8 Trainium2 NeuronCores.

Strategy (expert/token parallel):
  Tokens are sorted by expert; m_sizes gives each expert's contiguous row
  range of x.  The host splits the full token range into single-expert
  chunks (balanced m_sizes -> one expert per core) and each core computes
  y_chunk = x_chunk @ weight[e].T, with the host scattering chunk rows back
  into the full output.

  Inputs are cast to bf16 on the host (PE streams 1 column/cycle for bf16
  same as fp32r, but HBM traffic halves; K=1024 accumulation stays fp32 in
  PSUM so the result error is ~5e-3, well inside the 2e-2 gate).  The
  output is stored transposed (yT, bf16) and the host casts/transposes it
  back, halving the store traffic too.  Per-core HBM traffic ~10 MB vs a
  ~57 us matmul roofline, so the kernel is tensor-engine-bound.

  The matmul puts the weight column block [k=128, n=128] stationary and
  streams the token dim as the moving free dim, so the ragged per-expert
  token count (973..1065 here) is NOT padded up to a multiple of 128 --
  only to the SPMD-uniform m_pad.  PSUM limits the moving dim to 512 fp32
  per bank, so the token range is cut into ceil(m_pad/512) near-equal
  chunks (all >=256 wide, keeping LDWEIGHTS hidden under the stream).
"""

import math
import os

import ml_dtypes
import numpy as np

_N_CORES = 8
_P = 128
_PSUM_F = 512  # PSUM bank width in fp32 = max moving free dim per matmul

_program_cache = {}


def _m_chunks(m_pad):
    """Cut [0, m_pad) into chunks of <=512 (multiples of 8).

    The first chunk is small (256) so the first x DMA lands early and the
    PE can start the first accumulation group sooner; the rest near-equal.
    """
    if m_pad <= _PSUM_F:
        return [(0, m_pad)]
    bounds = [(0, 256)]
    rest = m_pad - 256
    n = (rest + _PSUM_F - 1) // _PSUM_F
    base = min(_PSUM_F, ((-(-rest // n) + 7) // 8) * 8)
    c = 256
    while c < m_pad:
        bounds.append((c, min(c + base, m_pad)))
        c = bounds[-1][1]
    return bounds


def _w_groups(nt_n):
    """Weight column-tile groups in consumption order: two singles, then
    doubling widths -- small DMAs early (low latency), big ones late."""
    groups = [(0, 1), (1, 2)] if nt_n > 1 else [(0, 1)]
    while groups[-1][1] < nt_n:
        g0 = groups[-1][1]
        groups.append((g0, min(2 * g0, nt_n)))
    return groups


def _build_program(m_pad, d_in, d_out, out_engine="scalar", warm_mms=10):
    import concourse.mybir as mybir
    import concourse.tile as tile
    from concourse import bacc

    kc_n = d_in // _P   # contraction chunks of 128
    nt_n = d_out // _P  # stationary weight column tiles
    chunks = _m_chunks(m_pad)
    groups = _w_groups(nt_n)

    # Hosts packs x as [p, ci, kc, m_chunk] and w as [p, g, kc, cols] so
    # every DMA is a plain 2D contiguous slice with multi-KB partition
    # lines (small-line ramp DMAs ran at ~150 GB/s, these hit line rate).
    x_off = {}
    off = 0
    for ci, (c0, c1) in enumerate(chunks):
        x_off[ci] = off
        off += kc_n * (c1 - c0)
    w_off = {}
    off = 0
    for gi, (g0, g1) in enumerate(groups):
        w_off[gi] = off
        off += kc_n * (g1 - g0) * _P

    nc = bacc.Bacc("TRN2", target_bir_lowering=False, debug=False)
    xT = nc.dram_tensor("xT", [_P, kc_n * m_pad], mybir.dt.bfloat16,
                        kind="ExternalInput")
    wT = nc.dram_tensor("wT", [_P, kc_n * d_out], mybir.dt.bfloat16,
                        kind="ExternalInput")
    yT = nc.dram_tensor("yT", [d_out, m_pad], mybir.dt.bfloat16,
                        kind="ExternalOutput")
    y3 = yT.rearrange("(nt p) m -> nt p m", p=_P)

    with tile.TileContext(nc) as tc:
        with (
            tc.tile_pool(name="xw", bufs=1) as xwpool,
            tc.tile_pool(name="out", bufs=16) as outpool,
            tc.tile_pool(name="psum", bufs=7, space="PSUM") as psumpool,
            tc.tile_pool(name="warm", bufs=1, space="PSUM") as warmpool,
        ):
            xall = xwpool.tile([_P, kc_n * m_pad], mybir.dt.bfloat16,
                               tag="xall", name="xall")
            wall = xwpool.tile([_P, kc_n * d_out], mybir.dt.bfloat16,
                               tag="wall", name="wall")

            # PE warm-up: dummy matmuls with no DMA dependency so the HAM
            # clock gate reaches 8/8 while the input ramp streams in.  DVE
            # memset (gpsimd's Q7 startup is multi-us).
            if warm_mms:
                scr = xwpool.tile([_P, 640], mybir.dt.bfloat16, tag="scr",
                                  name="scr")
                nc.vector.memset(scr[:], 0)
                psw = warmpool.tile([_P, _PSUM_F], mybir.dt.float32, tag="psw",
                                    name="psw")
                for i in range(warm_mms):
                    nc.tensor.matmul(psw[:], scr[:, 0:_P], scr[:, _P:640],
                                     start=(i == 0), stop=(i == warm_mms - 1))

            # DMA emission order == arrival order on the sync queue, in
            # consumption order with small transfers first (per-DMA
            # completion receipt ~1.5us sits on every dependency edge):
            # w n0, x c0, x c1, w n1, x c2.., then w groups of doubling size.
            def wslice(gi):
                e = w_off[gi + 1] if gi + 1 < len(groups) else kc_n * d_out
                return w_off[gi], e

            def xslice(ci):
                e = x_off[ci + 1] if ci + 1 < len(chunks) else kc_n * m_pad
                return x_off[ci], e

            order = [("w", 0), ("x", 0)]
            order += [("x", ci) for ci in range(1, len(chunks))]
            order += [("w", gi) for gi in range(1, len(groups))]
            for kind, i in order:
                s, e = wslice(i) if kind == "w" else xslice(i)
                src, dst = (wT, wall) if kind == "w" else (xT, xall)
                nc.sync.dma_start(dst[:, s:e], src[:, s:e])

            out_dma = {
                "gpsimd": nc.gpsimd.dma_start,
                "scalar": nc.scalar.dma_start,
                "sync": nc.sync.dma_start,
            }[out_engine]

            nt_group = {}
            for gi, (g0, g1) in enumerate(groups):
                for nt in range(g0, g1):
                    nt_group[nt] = (gi, g0, g1)

            for nt in range(nt_n):
                gi, g0, g1 = nt_group[nt]
                ps = [psumpool.tile([_P, _PSUM_F], mybir.dt.float32, tag="ps",
                                    name=f"ps{nt}_{ci}")
                      for ci in range(len(chunks))]
                o = outpool.tile([_P, m_pad], mybir.dt.bfloat16, tag="o")
                for ci, (c0, c1) in enumerate(chunks):
                    for kc in range(kc_n):
                        woff = (w_off[gi] + kc * (g1 - g0) * _P
                                + (nt - g0) * _P)
                        nc.tensor.matmul(
                            ps[ci][:, :c1 - c0],
                            wall[:, woff:woff + _P],
                            xall[:, x_off[ci] + kc * (c1 - c0):
                                 x_off[ci] + (kc + 1) * (c1 - c0)],
                            start=(kc == 0),
                            stop=(kc == kc_n - 1),
                        )
                    nc.vector.tensor_copy(o[:, c0:c1], ps[ci][:, :c1 - c0])
                    if nt >= nt_n - 2:
                        # per-chunk stores at the end shorten the drain tail
                        out_dma(y3[nt, :, c0:c1], o[:, c0:c1])
                if nt < nt_n - 2:
                    out_dma(y3[nt], o[:])
    nc.compile()
    return nc


def _pack_x(xT_b, chunks, kc_n):
    """[d_in, m_pad] bf16 -> [128, sum_ci kc_n*len_ci] in [ci][kc][m] order."""
    v = xT_b.reshape(kc_n, _P, -1).transpose(1, 0, 2)  # [p, kc, m]
    return np.concatenate(
        [np.ascontiguousarray(v[:, :, c0:c1]).reshape(_P, -1)
         for c0, c1 in chunks], axis=1)


def _pack_w(wT_b, groups, kc_n):
    """[d_in, d_out] bf16 -> [128, kc_n*d_out] in [g][kc][cols] order."""
    v = wT_b.reshape(kc_n, _P, -1).transpose(1, 0, 2)  # [p, kc, o]
    return np.concatenate(
        [np.ascontiguousarray(v[:, :, g0 * _P:g1 * _P]).reshape(_P, -1)
         for g0, g1 in groups], axis=1)


# Largest chunk one core handles per SPMD round (SBUF-bounded).
_MAX_CHUNK = 2560


def _plan_chunks(m_sizes, T):
    """Split [0, T) into single-expert chunks, balanced by length.

    Every chunk is <= _MAX_CHUNK rows.  Returns a list of (expert, row0,
    row1) padded with empty (0, 0, 0) chunks to a multiple of _N_CORES,
    or None if there are no rows at all.
    """
    off = np.cumsum(np.asarray(m_sizes, dtype=np.int64))
    starts = np.clip(np.concatenate([[0], off[:-1]]), 0, T)
    ends = np.clip(off, 0, T)
    segs = [(e, int(starts[e]), int(ends[e]))
            for e in range(len(m_sizes)) if ends[e] > starts[e]]
    if not segs:
        return None
    lens = np.array([s1 - s0 for _, s0, s1 in segs], dtype=np.float64)
    # mandatory splits so no chunk exceeds _MAX_CHUNK, then distribute any
    # spare cores (up to the next multiple of _N_CORES) to the biggest shares
    n_chunks = np.ceil(lens / _MAX_CHUNK).astype(np.int64)
    total = int(n_chunks.sum())
    spare = (-total) % _N_CORES if total > _N_CORES else _N_CORES - total
    for _ in range(spare):
        i = int(np.argmax(lens / n_chunks))
        n_chunks[i] += 1
    chunks = []
    for (e, s0, s1), k in zip(segs, n_chunks):
        L = s1 - s0
        bounds = [s0 + (L * i) // k for i in range(int(k) + 1)]
        for i in range(int(k)):
            if bounds[i + 1] > bounds[i]:
                chunks.append((e, bounds[i], bounds[i + 1]))
    while len(chunks) % _N_CORES:
        chunks.append((0, 0, 0))
    return chunks


def kernel(x, weight, m_sizes):
    from concourse.bass_utils import run_bass_kernel_spmd

    x = np.ascontiguousarray(np.asarray(x), dtype=np.float32)
    weight = np.asarray(weight, dtype=np.float32)
    m_arr = np.asarray(m_sizes)

    T, d_in = x.shape
    E, d_out, _ = weight.shape

    y = np.zeros((T, d_out), dtype=np.float32)
    chunks = _plan_chunks(m_arr, T)
    if chunks is None:
        return y

    max_len = max(r1 - r0 for _, r0, r1 in chunks)
    m_pad = max(_P, int(math.ceil(max_len / 4)) * 4)

    out_engine = os.environ.get("MOE_OUT_ENGINE", "scalar")
    warm_mms = int(os.environ.get("MOE_WARM_MMS", "14"))
    key = (m_pad, d_in, d_out, out_engine, warm_mms)
    if key not in _program_cache:
        _program_cache[key] = _build_program(m_pad, d_in, d_out, out_engine,
                                             warm_mms)
    nc = _program_cache[key]

    bf16 = ml_dtypes.bfloat16
    kc_n = d_in // _P
    m_chunks = _m_chunks(m_pad)
    groups = _w_groups(d_out // _P)
    # weight[e].T packed, built once per expert actually used
    wT_cache = {}
    for round0 in range(0, len(chunks), _N_CORES):
        batch = chunks[round0:round0 + _N_CORES]
        in_maps = []
        for e, r0, r1 in batch:
            xT = np.zeros((d_in, m_pad), dtype=bf16)
            if r1 > r0:
                xT[:, : r1 - r0] = x[r0:r1].T.astype(bf16)
            if e not in wT_cache:
                wT_cache[e] = _pack_w(weight[e].T.astype(bf16), groups, kc_n)
            in_maps.append({"xT": _pack_x(xT, m_chunks, kc_n),
                            "wT": wT_cache[e]})

        res = run_bass_kernel_spmd(nc, in_maps, core_ids=list(range(_N_CORES)))

        for (e, r0, r1), out in zip(batch, res.results):
            if r1 > r0:
                y[r0:r1] = out["yT"][:, : r1 - r0].T.astype(np.float32)
    return y
